# revision 25
# baseline (speedup 1.0000x reference)
"""DEDICOM decoder forward on 8 Trainium2 NeuronCores.

score = sigmoid((z_i * (z_j @ R.T)) @ (D*D).T)

Data-parallel over batch: each core handles B/8 = 4096 rows.

Fast path (constant D, as produced by setup_inputs where D == ones):
  (D*D).T is a constant matrix c = d^2, so
    score[b, r] = sigmoid(sum_h (d^2 * z_i[b,h]) * (z_j @ R.T)[b,h])  for all r
  i.e. the output is rank-1 along r: ONE f32 per batch row. d^2 is folded
  into z_i on the host.
  Per core dataflow (batch rows on partitions):
    - MM1 (bf16): Rzj[b, h'] = sum_h z_j[b,h] * R^T[h,h']
        lhsT = z_j^T chunk [128h x 128b] stationary, rhs = R^T [128h x 512h'].
    - DVE tensor_tensor_reduce: s[b] = sum_h' zi_scaled[b,h'] * Rzj[b,h']
      (multiply + rowsum in ONE DVE instr per 128-row chunk; ACT idle)
    - ACT: sig[b] = Sigmoid(s[b])  ([128, nm] per tile)
    - out: [128, 32] f32 per core (16 KB); host expands rows to 960 cols.
  All input DMAs are per-partition CONTIGUOUS: the host pre-packs z_i,
  z_j^T and R^T into [128, *] arrays laid out tile-by-tile so each load is
  one 2-4 KB descriptor per partition.
  HBM traffic/core: 8.0 MB bf16 z in + 0.5 MB R + 16 KB out (vs 12.3 MB
  for the u16-broadcast version and 35 MB for the general path).

General path (non-constant D): original f32r kernel, kept as fallback.
"""
import sys

sys.path.insert(0, "/opt/trn_rl_repo")

import numpy as np  # noqa: E402

B = 32768
H = 512  # hidden
R_SE = 960  # num relation types
N_CORES = 8
BS = B // N_CORES  # 4096 batch rows per core
BT = 512  # batch tile
NM = BT // 128  # 4 b-128 chunks per tile
NK = H // 128  # 4 h-chunks
NT = BS // BT  # 8 batch tiles per core
RH = R_SE // 2  # 480, moving-dim half for MM2 (general path)
NCH = BS // 128  # 32 b-128 chunks per core (fast-path output cols)
# fast-path tile schedule: two small tiles first so the PE pipeline can
# start as soon as the first 0.25 MB of z_j^T lands
SIZES = [256, 256] + [BT] * (NT - 1)
OFFS = [sum(SIZES[:i]) for i in range(len(SIZES))]

_compiled_fast = None
_compiled_general = None


def _build_fast():
    import concourse.tile as tile
    import concourse.mybir as mybir
    from concourse import bacc

    f32 = mybir.dt.float32
    bf16 = mybir.dt.bfloat16
    mult = mybir.AluOpType.mult
    add = mybir.AluOpType.add
    Sigmoid = mybir.ActivationFunctionType.Sigmoid
    Copy = mybir.ActivationFunctionType.Copy

    nc = bacc.Bacc("TRN2", target_bir_lowering=False, debug=False)
    # host-packed layouts: per-partition contiguous, tile-by-tile (see
    # _pack_* helpers).  zi has d^2 folded in.
    zi_d = nc.dram_tensor("zi", [128, NCH * H], bf16, kind="ExternalInput").ap()
    zjt_d = nc.dram_tensor("zjt", [128, NK * BS], bf16, kind="ExternalInput").ap()
    rt_d = nc.dram_tensor("rt", [128, NK * H], bf16, kind="ExternalInput").ap()
    out_d = nc.dram_tensor("out", [128, NCH], f32, kind="ExternalOutput").ap()

    with tile.TileContext(nc) as tc:
        with (
            tc.tile_pool(name="const", bufs=1) as const,
            tc.tile_pool(name="zjt", bufs=6) as zjp,
            tc.tile_pool(name="zi", bufs=6) as zip_,
            tc.tile_pool(name="qd", bufs=4) as qdp,
            tc.tile_pool(name="sc", bufs=4) as scp,
            tc.tile_pool(name="sg", bufs=4) as sgp,
            tc.tile_pool(name="ps", bufs=6, space="PSUM") as psp,
            tc.tile_pool(name="warm", bufs=1, space="PSUM") as warmp,
        ):
            # rt + zjt tile 0 ride the gpsimd SWDGE: its engine preamble ends
            # ~1.5us before the sync HWDGE ring's first transfer, so the
            # train-start data lands earlier
            rt_r = const.tile([128, NK, H], bf16, tag="rt_r")
            nc.gpsimd.dma_start(rt_r[:], rt_d.rearrange("p (k n) -> p k n", n=H))

            # PE warmup during the initial DMA wait: junk matmuls on a zeroed
            # scratch tile start the HAM clock-gate activity window early.
            # Just a few — the real matmuls arrive ~2us later and burn the
            # rest of the ~3.4us cold window themselves.
            warm_sb = const.tile([128, BT], bf16, tag="warm_sb")
            nc.vector.memset(warm_sb[:], 0.0)
            warm_ps = warmp.tile([128, H], f32, tag="warm_ps")
            for _ in range(2):
                nc.tensor.matmul(
                    warm_ps[:], warm_sb[:, :128], warm_sb[:], start=True, stop=True
                )

            coff = 0  # output column (b-128 chunk) offset
            for t, (b0, bt) in enumerate(zip(OFFS, SIZES)):
                nm = bt // 128
                zjt_r = zjp.tile([128, NK, bt], bf16, tag="zjt", name=f"zjt_{t}")
                zjt_engine = nc.gpsimd if t == 0 else nc.sync
                zjt_engine.dma_start(
                    zjt_r[:],
                    zjt_d[:, NK * b0 : NK * (b0 + bt)].rearrange(
                        "p (k b) -> p k b", b=bt
                    ),
                )

                # zi rides the SAME sync ring, right behind its tile's zjt:
                # ring order == need order, and the scalar ring stays free of
                # DMAs so sigmoid completion never gates a load issue.
                zi_r = zip_.tile([128, nm, H], bf16, tag="zi", name=f"zi_{t}")
                nc.sync.dma_start(
                    zi_r[:],
                    zi_d[:, coff * H : (coff + nm) * H].rearrange(
                        "p (m h) -> p m h", h=H
                    ),
                )
                s_cols = scp.tile([128, nm], f32, tag="sc", name=f"s_{t}")
                for m in range(nm):
                    ps = psp.tile([128, H], f32, tag="ps", name=f"ps_{t}_{m}")
                    for k in range(NK):
                        nc.tensor.matmul(
                            ps[:],
                            zjt_r[:, k, m * 128 : (m + 1) * 128],
                            rt_r[:, k, :],
                            start=(k == 0),
                            stop=(k == NK - 1),
                        )
                    # s[b] = sum_h zi[b,h] * Rzj[b,h]: ONE custom-DVE pass
                    # (production-proven opcode incl. accum_out; the native
                    # TENSOR_TENSOR_REDUCE ISA op crashes on HW). qd is a
                    # dead store.
                    qd = qdp.tile([128, H], bf16, tag="qd", name=f"qd_{t}_{m}")
                    nc.vector.affine_mul_reduce(
                        out=qd[:],
                        accum_out=s_cols[:, m : m + 1],
                        in0=ps[:],
                        in1=zi_r[:, m, :],
                        scale=1.0,
                        bias=0.0,
                    )
                # one sigmoid per tile over the per-chunk sums, straight to
                # a tiny [128, nm] store (output is rank-1 along r)
                sg_t = sgp.tile([128, nm], f32, tag="sg", name=f"sg_{t}")
                nc.scalar.activation(sg_t[:], s_cols[:], Sigmoid)
                if t < len(SIZES) - 1:
                    # tiny stores ride the otherwise-idle gpsimd SWDGE so
                    # they never occupy the load rings
                    nc.gpsimd.dma_start(out_d[:, coff : coff + nm], sg_t[:])
                else:
                    # tail store: sync ring is idle by now and HWDGE has the
                    # shorter completion latency
                    nc.sync.dma_start(out_d[:, coff : coff + nm], sg_t[:])
                coff += nm

    nc.compile()
    return nc


def _build_general():
    import concourse.tile as tile
    import concourse.mybir as mybir
    from concourse import bacc

    f32 = mybir.dt.float32
    f32r = mybir.dt.float32r

    nc = bacc.Bacc("TRN2", target_bir_lowering=False, debug=False)
    # transposed layouts [h, b]; zjt/rt/d2t pre-rounded to the f32r grid on
    # host so they can be DMA'd straight into float32r tiles (the walrus
    # verifier requires f32r matmul inputs to come from a rounding producer)
    zit_d = nc.dram_tensor("zit", [H, BS], f32, kind="ExternalInput").ap()
    zjt_d = nc.dram_tensor("zjt", [H, BS], f32r, kind="ExternalInput").ap()
    rt_d = nc.dram_tensor("rt", [H, H], f32r, kind="ExternalInput").ap()  # R.T
    d2t_d = nc.dram_tensor("d2t", [H, R_SE], f32r, kind="ExternalInput").ap()
    out_d = nc.dram_tensor("out", [BS, R_SE], f32, kind="ExternalOutput").ap()

    with tile.TileContext(nc) as tc:
        with (
            tc.tile_pool(name="const", bufs=1) as const,
            tc.tile_pool(name="zt", bufs=4) as ztp,
            tc.tile_pool(name="qp", bufs=2) as qp,
            tc.tile_pool(name="sig", bufs=6) as sigp,
            tc.tile_pool(name="ps1", bufs=3, space="PSUM") as ps1p,
            tc.tile_pool(name="ps2", bufs=4, space="PSUM") as ps2p,
            tc.tile_pool(name="warm", bufs=1, space="PSUM") as warmp,
        ):
            rt_r = const.tile([128, NK, H], f32r, tag="rt_r")
            nc.sync.dma_start(rt_r[:], rt_d.rearrange("(k p) n -> p k n", p=128))

            # PE warmup during the initial DMA wait
            warm_f = const.tile([128, BT], f32, tag="warm_f")
            nc.vector.memset(warm_f[:], 0.0)
            warm_sb = const.tile([128, BT], f32r, tag="warm_sb")
            nc.vector.tensor_copy(warm_sb[:], warm_f[:])
            warm_ps = warmp.tile([128, BT], f32, tag="warm_ps")
            for _ in range(10):
                nc.tensor.matmul(
                    warm_ps[:], warm_sb[:, :128], warm_sb[:], start=True, stop=True
                )

            d2t_r = const.tile([128, NK, R_SE], f32r, tag="d2t_r")

            sizes = [256, 256] + [512] * (NT - 1)
            offs = [sum(sizes[:i]) for i in range(len(sizes))]
            tiles = list(zip(offs, sizes))
            for t, (b0, bt) in enumerate(tiles):
                nm = bt // 128
                zjt_r = ztp.tile([128, NK, bt], f32r, tag="zjt", name=f"zjt_{t}")
                nc.sync.dma_start(
                    zjt_r[:],
                    zjt_d[:, b0 : b0 + bt].rearrange("(k p) b -> p k b", p=128),
                )
                if t == 0:
                    # d2t is first needed by MM2 of tile 0; slot its halves
                    # right behind tile 0's zjt in the HWDGE queue
                    nc.sync.dma_start(
                        d2t_r[:, :, 0:RH],
                        d2t_d[:, 0:RH].rearrange("(k p) n -> p k n", p=128),
                    )
                zit_f = ztp.tile([128, NK, bt], f32, tag="zit", name=f"zit_{t}")
                nc.sync.dma_start(
                    zit_f[:],
                    zit_d[:, b0 : b0 + bt].rearrange("(k p) b -> p k b", p=128),
                )
                if t == 0:
                    nc.sync.dma_start(
                        d2t_r[:, :, RH:R_SE],
                        d2t_d[:, RH:R_SE].rearrange("(k p) n -> p k n", p=128),
                    )

                # MM1 + q per h'-chunk j
                q_r = qp.tile([128, NK, bt], f32r, tag="q", name=f"q_{t}")
                for j in range(NK):
                    p1 = ps1p.tile([128, bt], f32, tag="ps1", name=f"p1_{t}_{j}")
                    for k in range(NK):
                        nc.tensor.matmul(
                            p1[:],
                            rt_r[:, k, j * 128 : (j + 1) * 128],
                            zjt_r[:, k, :],
                            start=(k == 0),
                            stop=(k == NK - 1),
                        )
                    nc.vector.tensor_mul(q_r[:, j, :], p1[:], zit_f[:, j, :])

                # MM2 + sigmoid + store per b-128 chunk m
                last_tile = t == len(tiles) - 1
                for m in range(nm):
                    sg = sigp.tile([128, R_SE], f32, tag="sg", name=f"sg_{t}_{m}")
                    for rh in range(2):
                        p2 = ps2p.tile([128, RH], f32, tag="ps2", name=f"p2_{t}_{m}_{rh}")
                        for k in range(NK):
                            nc.tensor.matmul(
                                p2[:],
                                q_r[:, k, m * 128 : (m + 1) * 128],
                                d2t_r[:, k, rh * RH : (rh + 1) * RH],
                                start=(k == 0),
                                stop=(k == NK - 1),
                            )
                        nc.scalar.activation(
                            sg[:, rh * RH : (rh + 1) * RH],
                            p2[:],
                            mybir.ActivationFunctionType.Sigmoid,
                        )
                        if last_tile:
                            # tail: half-stores via the (now idle) HWDGE queue
                            nc.sync.dma_start(
                                out_d[
                                    b0 + m * 128 : b0 + (m + 1) * 128,
                                    rh * RH : (rh + 1) * RH,
                                ],
                                sg[:, rh * RH : (rh + 1) * RH],
                            )
                    if not last_tile:
                        nc.gpsimd.dma_start(
                            out_d[b0 + m * 128 : b0 + (m + 1) * 128, :], sg[:]
                        )

    nc.compile()
    return nc


def _get_fast():
    global _compiled_fast
    if _compiled_fast is None:
        _compiled_fast = _build_fast()
    return _compiled_fast


def _get_general():
    global _compiled_general
    if _compiled_general is None:
        _compiled_general = _build_general()
    return _compiled_general


def _round_f32r(x: np.ndarray) -> np.ndarray:
    """Round fp32 to the f32r grid (12 dropped mantissa bits, round-nearest).
    Values on the grid are fixed points of the hardware's own rounding."""
    b = np.ascontiguousarray(x, dtype=np.float32).view(np.uint32)
    r = (b + 0x800 + ((b >> 12) & 1)) & np.uint32(0xFFFFF000)
    return r.view(np.float32)


def _pack_zi(zi_core):
    """[BS, H] -> [128, NCH*H]: tile-by-tile, row b0+m*128+p lands in
    partition p, columns (coff+m)*H : (coff+m+1)*H — contiguous per line."""
    blocks = []
    for b0, bt in zip(OFFS, SIZES):
        nm = bt // 128
        blk = zi_core[b0 : b0 + bt].reshape(nm, 128, H)
        blocks.append(blk.transpose(1, 0, 2).reshape(128, nm * H))
    return np.ascontiguousarray(np.concatenate(blocks, axis=1))


def _pack_zjt(zjt_core):
    """[H, BS] -> [128, NK*BS]: tile-by-tile, h=k*128+p lands in partition
    p; per tile the NK k-blocks of bt columns are contiguous per line."""
    blocks = []
    for b0, bt in zip(OFFS, SIZES):
        blk = zjt_core[:, b0 : b0 + bt].reshape(NK, 128, bt)
        blocks.append(blk.transpose(1, 0, 2).reshape(128, NK * bt))
    return np.ascontiguousarray(np.concatenate(blocks, axis=1))


def _kernel_fast(z_i, z_j, R, D):
    import ml_dtypes
    from concourse import bass_utils

    nc = _get_fast()
    bf16 = ml_dtypes.bfloat16

    d2 = np.float32(np.float64(D.flat[0]) ** 2)
    zi_f = np.asarray(z_i, dtype=np.float32)
    if d2 != 1.0:
        zi_f = zi_f * d2  # fold d^2 into z_i (score = sigmoid(d2 * zi.Rzj))
    zi_b = zi_f.astype(bf16)  # [B, H]
    zjt_b = np.asarray(z_j, dtype=np.float32).T.astype(bf16)  # [H, B]
    rt_b = np.asarray(R, dtype=np.float32).T.astype(bf16)  # [H, H]
    rt_p = np.ascontiguousarray(
        rt_b.reshape(NK, 128, H).transpose(1, 0, 2).reshape(128, NK * H)
    )

    in_maps = []
    for c in range(N_CORES):
        sl = slice(c * BS, (c + 1) * BS)
        in_maps.append(
            {
                "zi": _pack_zi(zi_b[sl]),
                "zjt": _pack_zjt(zjt_b[:, sl]),
                "rt": rt_p,
            }
        )

    res = bass_utils.run_bass_kernel_spmd(nc, in_maps, core_ids=list(range(N_CORES)))
    global last_result
    last_result = res
    out = np.empty((B, R_SE), dtype=np.float32)
    for c in range(N_CORES):
        sig = np.asarray(res.results[c]["out"])  # [128, NCH], sig[p,ch]=row ch*128+p
        rows = np.ascontiguousarray(sig.T).reshape(BS)
        out[c * BS : (c + 1) * BS, :] = rows[:, None]
    return out


def _kernel_general(z_i, z_j, R, D):
    from concourse import bass_utils

    nc = _get_general()

    z_i = np.asarray(z_i, dtype=np.float32)
    z_j = np.asarray(z_j, dtype=np.float32)
    zit = np.ascontiguousarray(z_i.T)  # [H, B]
    zjt = _round_f32r(np.ascontiguousarray(z_j.T))
    rt = _round_f32r(np.asarray(R, dtype=np.float32).T)
    d2 = np.asarray(D, dtype=np.float32)
    d2t = _round_f32r((d2 * d2).T)

    in_maps = []
    for c in range(N_CORES):
        sl = slice(c * BS, (c + 1) * BS)
        in_maps.append(
            {
                "zit": np.ascontiguousarray(zit[:, sl]),
                "zjt": np.ascontiguousarray(zjt[:, sl]),
                "rt": rt,
                "d2t": d2t,
            }
        )

    res = bass_utils.run_bass_kernel_spmd(nc, in_maps, core_ids=list(range(N_CORES)))
    global last_result
    last_result = res
    out = np.empty((B, R_SE), dtype=np.float32)
    for c in range(N_CORES):
        out[c * BS : (c + 1) * BS] = res.results[c]["out"]
    return out


def kernel(z_i: np.ndarray, z_j: np.ndarray, R: np.ndarray, D: np.ndarray, **extra):
    D = np.asarray(D)
    if D.size and np.all(D == D.flat[0]):
        return _kernel_fast(z_i, z_j, R, D)
    return _kernel_general(z_i, z_j, R, D)


last_result = None


def _install_ntff_shim():
    """Provide antenv.axon_hooks (absent from this image) so that
    run_bass_kernel_spmd(trace=True) can capture NTFF profiles through
    the axon PJRT .so. No-op if anything is missing."""
    import types
    import contextlib
    import ctypes

    try:
        import antenv
        import antenv.axon_hooks  # noqa: F401

        return  # already present
    except ImportError:
        pass

    so_path = "/opt/axon/libaxon_pjrt.so"
    try:
        lib = ctypes.CDLL(so_path)
    except OSError:
        return
    if not hasattr(lib, "axon_start_nrt_profile"):
        return
    lib.axon_start_nrt_profile.argtypes = [
        ctypes.POINTER(ctypes.c_int64),
        ctypes.c_size_t,
    ]
    lib.axon_start_nrt_profile.restype = ctypes.c_int64
    lib.axon_stop_nrt_profile.argtypes = [ctypes.c_char_p]
    lib.axon_stop_nrt_profile.restype = ctypes.c_int64

    @contextlib.contextmanager
    def _hook(output_dir, device_ids):
        import jax

        jax.devices()
        if device_ids:
            ids = (ctypes.c_int64 * len(device_ids))(*device_ids)
            rc = lib.axon_start_nrt_profile(ids, len(device_ids))
        else:
            rc = lib.axon_start_nrt_profile(None, 0)
        if rc != 0:
            raise RuntimeError(f"axon_start_nrt_profile rc={rc}")
        try:
            yield
        finally:
            n = lib.axon_stop_nrt_profile(str(output_dir).encode())
            print(f"ntff profile: {n} file(s) written to {output_dir}", file=sys.stderr)

    mod = types.ModuleType("antenv.axon_hooks")
    mod.get_axon_ntff_profile_hook = lambda: _hook
    mod.set_axon_ntff_profile_hook = lambda h: None
    sys.modules["antenv.axon_hooks"] = mod
    antenv.axon_hooks = mod


_install_ntff_shim()



# revision 29
# speedup vs baseline: 1.1076x; 1.1076x over previous
"""DEDICOM decoder forward on 8 Trainium2 NeuronCores.

score = sigmoid((z_i * (z_j @ R.T)) @ (D*D).T)

Data-parallel over batch: each core handles B/8 = 4096 rows.

Fast path (constant D, as produced by setup_inputs where D == ones):
  (D*D).T is a constant matrix c = d^2, so
    score[b, r] = sigmoid(sum_h (d^2 * z_i[b,h]) * (z_j @ R.T)[b,h])  for all r
  i.e. the output is rank-1 along r: ONE f32 per batch row. d^2 is folded
  into z_i on the host.
  Per core dataflow (batch rows on partitions):
    - MM1 (bf16): Rzj[b, h'] = sum_h z_j[b,h] * R^T[h,h']
        lhsT = z_j^T chunk [128h x 128b] stationary, rhs = R^T [128h x 512h'].
    - DVE tensor_tensor_reduce: s[b] = sum_h' zi_scaled[b,h'] * Rzj[b,h']
      (multiply + rowsum in ONE DVE instr per 128-row chunk; ACT idle)
    - ACT: sig[b] = Sigmoid(s[b])  ([128, nm] per tile)
    - out: [128, 32] f32 per core (16 KB); host expands rows to 960 cols.
  All input DMAs are per-partition CONTIGUOUS: the host pre-packs z_i,
  z_j^T and R^T into [128, *] arrays laid out tile-by-tile so each load is
  one 2-4 KB descriptor per partition.
  HBM traffic/core: 8.0 MB bf16 z in + 0.5 MB R + 16 KB out (vs 12.3 MB
  for the u16-broadcast version and 35 MB for the general path).

General path (non-constant D): original f32r kernel, kept as fallback.
"""
import sys

sys.path.insert(0, "/opt/trn_rl_repo")

import numpy as np  # noqa: E402

B = 32768
H = 512  # hidden
R_SE = 960  # num relation types
N_CORES = 8
BS = B // N_CORES  # 4096 batch rows per core
BT = 512  # batch tile
NM = BT // 128  # 4 b-128 chunks per tile
NK = H // 128  # 4 h-chunks
NT = BS // BT  # 8 batch tiles per core
RH = R_SE // 2  # 480, moving-dim half for MM2 (general path)
NCH = BS // 128  # 32 b-128 chunks per core (fast-path output cols)
# fast-path tile schedule: two small tiles first so the PE pipeline can
# start as soon as the first 0.25 MB of z_j^T lands
SIZES = [256, 256] + [BT] * (NT - 1)
OFFS = [sum(SIZES[:i]) for i in range(len(SIZES))]

_compiled_fast = None
_compiled_general = None


def _build_fast():
    import concourse.tile as tile
    import concourse.mybir as mybir
    from concourse import bacc

    f32 = mybir.dt.float32
    bf16 = mybir.dt.bfloat16
    mult = mybir.AluOpType.mult
    add = mybir.AluOpType.add
    Sigmoid = mybir.ActivationFunctionType.Sigmoid
    Copy = mybir.ActivationFunctionType.Copy

    nc = bacc.Bacc("TRN2", target_bir_lowering=False, debug=False)
    # host-packed layouts: per-partition contiguous, tile-by-tile (see
    # _pack_* helpers).  zi has d^2 folded in.
    zi_d = nc.dram_tensor("zi", [128, NCH * H], bf16, kind="ExternalInput").ap()
    zjt_d = nc.dram_tensor("zjt", [128, NK * BS], bf16, kind="ExternalInput").ap()
    rt_d = nc.dram_tensor("rt", [128, NK * H], bf16, kind="ExternalInput").ap()
    out_d = nc.dram_tensor("out", [128, NCH], f32, kind="ExternalOutput").ap()

    with tile.TileContext(nc) as tc:
        with (
            tc.tile_pool(name="const", bufs=1) as const,
            tc.tile_pool(name="zjt", bufs=6) as zjp,
            tc.tile_pool(name="zi", bufs=6) as zip_,
            tc.tile_pool(name="qd", bufs=4) as qdp,
            tc.tile_pool(name="sc", bufs=4) as scp,
            tc.tile_pool(name="sg", bufs=4) as sgp,
            tc.tile_pool(name="ps", bufs=6, space="PSUM") as psp,
            tc.tile_pool(name="warm", bufs=1, space="PSUM") as warmp,
        ):
            rt_r = const.tile([128, NK, H], bf16, tag="rt_r")
            nc.sync.dma_start(rt_r[:], rt_d.rearrange("p (k n) -> p k n", n=H))

            # PE warmup during the initial DMA wait: junk matmuls on a zeroed
            # scratch tile start the HAM clock-gate activity window early.
            # Just a few — the real matmuls arrive ~2us later and burn the
            # rest of the ~3.4us cold window themselves.
            warm_sb = const.tile([128, BT], bf16, tag="warm_sb")
            nc.vector.memset(warm_sb[:], 0.0)
            warm_ps = warmp.tile([128, H], f32, tag="warm_ps")
            for _ in range(8):
                nc.tensor.matmul(
                    warm_ps[:], warm_sb[:, :128], warm_sb[:], start=True, stop=True
                )

            coff = 0  # output column (b-128 chunk) offset
            for t, (b0, bt) in enumerate(zip(OFFS, SIZES)):
                nm = bt // 128
                zjt_r = zjp.tile([128, NK, bt], bf16, tag="zjt", name=f"zjt_{t}")
                nc.sync.dma_start(
                    zjt_r[:],
                    zjt_d[:, NK * b0 : NK * (b0 + bt)].rearrange(
                        "p (k b) -> p k b", b=bt
                    ),
                )

                # zi rides the SAME sync ring, right behind its tile's zjt:
                # ring order == need order, and the scalar ring stays free of
                # DMAs so sigmoid completion never gates a load issue.
                zi_r = zip_.tile([128, nm, H], bf16, tag="zi", name=f"zi_{t}")
                nc.sync.dma_start(
                    zi_r[:],
                    zi_d[:, coff * H : (coff + nm) * H].rearrange(
                        "p (m h) -> p m h", h=H
                    ),
                )
                s_cols = scp.tile([128, nm], f32, tag="sc", name=f"s_{t}")
                for m in range(nm):
                    ps = psp.tile([128, H], f32, tag="ps", name=f"ps_{t}_{m}")
                    for k in range(NK):
                        nc.tensor.matmul(
                            ps[:],
                            zjt_r[:, k, m * 128 : (m + 1) * 128],
                            rt_r[:, k, :],
                            start=(k == 0),
                            stop=(k == NK - 1),
                        )
                    # s[b] = sum_h zi[b,h] * Rzj[b,h]: ONE custom-DVE pass
                    # (production-proven opcode incl. accum_out; the native
                    # TENSOR_TENSOR_REDUCE ISA op crashes on HW). qd is a
                    # dead store.
                    qd = qdp.tile([128, H], bf16, tag="qd", name=f"qd_{t}_{m}")
                    nc.vector.affine_mul_reduce(
                        out=qd[:],
                        accum_out=s_cols[:, m : m + 1],
                        in0=ps[:],
                        in1=zi_r[:, m, :],
                        scale=1.0,
                        bias=0.0,
                    )
                # one sigmoid per tile over the per-chunk sums, straight to
                # a tiny [128, nm] store (output is rank-1 along r)
                sg_t = sgp.tile([128, nm], f32, tag="sg", name=f"sg_{t}")
                nc.scalar.activation(sg_t[:], s_cols[:], Sigmoid)
                # tiny stores ride the scalar HWDGE ring right behind their
                # sigmoid; gpsimd stays completely unused (cheaper init?)
                nc.scalar.dma_start(out_d[:, coff : coff + nm], sg_t[:])
                coff += nm

    nc.compile()
    return nc


def _build_general():
    import concourse.tile as tile
    import concourse.mybir as mybir
    from concourse import bacc

    f32 = mybir.dt.float32
    f32r = mybir.dt.float32r

    nc = bacc.Bacc("TRN2", target_bir_lowering=False, debug=False)
    # transposed layouts [h, b]; zjt/rt/d2t pre-rounded to the f32r grid on
    # host so they can be DMA'd straight into float32r tiles (the walrus
    # verifier requires f32r matmul inputs to come from a rounding producer)
    zit_d = nc.dram_tensor("zit", [H, BS], f32, kind="ExternalInput").ap()
    zjt_d = nc.dram_tensor("zjt", [H, BS], f32r, kind="ExternalInput").ap()
    rt_d = nc.dram_tensor("rt", [H, H], f32r, kind="ExternalInput").ap()  # R.T
    d2t_d = nc.dram_tensor("d2t", [H, R_SE], f32r, kind="ExternalInput").ap()
    out_d = nc.dram_tensor("out", [BS, R_SE], f32, kind="ExternalOutput").ap()

    with tile.TileContext(nc) as tc:
        with (
            tc.tile_pool(name="const", bufs=1) as const,
            tc.tile_pool(name="zt", bufs=4) as ztp,
            tc.tile_pool(name="qp", bufs=2) as qp,
            tc.tile_pool(name="sig", bufs=6) as sigp,
            tc.tile_pool(name="ps1", bufs=3, space="PSUM") as ps1p,
            tc.tile_pool(name="ps2", bufs=4, space="PSUM") as ps2p,
            tc.tile_pool(name="warm", bufs=1, space="PSUM") as warmp,
        ):
            rt_r = const.tile([128, NK, H], f32r, tag="rt_r")
            nc.sync.dma_start(rt_r[:], rt_d.rearrange("(k p) n -> p k n", p=128))

            # PE warmup during the initial DMA wait
            warm_f = const.tile([128, BT], f32, tag="warm_f")
            nc.vector.memset(warm_f[:], 0.0)
            warm_sb = const.tile([128, BT], f32r, tag="warm_sb")
            nc.vector.tensor_copy(warm_sb[:], warm_f[:])
            warm_ps = warmp.tile([128, BT], f32, tag="warm_ps")
            for _ in range(10):
                nc.tensor.matmul(
                    warm_ps[:], warm_sb[:, :128], warm_sb[:], start=True, stop=True
                )

            d2t_r = const.tile([128, NK, R_SE], f32r, tag="d2t_r")

            sizes = [256, 256] + [512] * (NT - 1)
            offs = [sum(sizes[:i]) for i in range(len(sizes))]
            tiles = list(zip(offs, sizes))
            for t, (b0, bt) in enumerate(tiles):
                nm = bt // 128
                zjt_r = ztp.tile([128, NK, bt], f32r, tag="zjt", name=f"zjt_{t}")
                nc.sync.dma_start(
                    zjt_r[:],
                    zjt_d[:, b0 : b0 + bt].rearrange("(k p) b -> p k b", p=128),
                )
                if t == 0:
                    # d2t is first needed by MM2 of tile 0; slot its halves
                    # right behind tile 0's zjt in the HWDGE queue
                    nc.sync.dma_start(
                        d2t_r[:, :, 0:RH],
                        d2t_d[:, 0:RH].rearrange("(k p) n -> p k n", p=128),
                    )
                zit_f = ztp.tile([128, NK, bt], f32, tag="zit", name=f"zit_{t}")
                nc.sync.dma_start(
                    zit_f[:],
                    zit_d[:, b0 : b0 + bt].rearrange("(k p) b -> p k b", p=128),
                )
                if t == 0:
                    nc.sync.dma_start(
                        d2t_r[:, :, RH:R_SE],
                        d2t_d[:, RH:R_SE].rearrange("(k p) n -> p k n", p=128),
                    )

                # MM1 + q per h'-chunk j
                q_r = qp.tile([128, NK, bt], f32r, tag="q", name=f"q_{t}")
                for j in range(NK):
                    p1 = ps1p.tile([128, bt], f32, tag="ps1", name=f"p1_{t}_{j}")
                    for k in range(NK):
                        nc.tensor.matmul(
                            p1[:],
                            rt_r[:, k, j * 128 : (j + 1) * 128],
                            zjt_r[:, k, :],
                            start=(k == 0),
                            stop=(k == NK - 1),
                        )
                    nc.vector.tensor_mul(q_r[:, j, :], p1[:], zit_f[:, j, :])

                # MM2 + sigmoid + store per b-128 chunk m
                last_tile = t == len(tiles) - 1
                for m in range(nm):
                    sg = sigp.tile([128, R_SE], f32, tag="sg", name=f"sg_{t}_{m}")
                    for rh in range(2):
                        p2 = ps2p.tile([128, RH], f32, tag="ps2", name=f"p2_{t}_{m}_{rh}")
                        for k in range(NK):
                            nc.tensor.matmul(
                                p2[:],
                                q_r[:, k, m * 128 : (m + 1) * 128],
                                d2t_r[:, k, rh * RH : (rh + 1) * RH],
                                start=(k == 0),
                                stop=(k == NK - 1),
                            )
                        nc.scalar.activation(
                            sg[:, rh * RH : (rh + 1) * RH],
                            p2[:],
                            mybir.ActivationFunctionType.Sigmoid,
                        )
                        if last_tile:
                            # tail: half-stores via the (now idle) HWDGE queue
                            nc.sync.dma_start(
                                out_d[
                                    b0 + m * 128 : b0 + (m + 1) * 128,
                                    rh * RH : (rh + 1) * RH,
                                ],
                                sg[:, rh * RH : (rh + 1) * RH],
                            )
                    if not last_tile:
                        nc.gpsimd.dma_start(
                            out_d[b0 + m * 128 : b0 + (m + 1) * 128, :], sg[:]
                        )

    nc.compile()
    return nc


def _get_fast():
    global _compiled_fast
    if _compiled_fast is None:
        _compiled_fast = _build_fast()
    return _compiled_fast


def _get_general():
    global _compiled_general
    if _compiled_general is None:
        _compiled_general = _build_general()
    return _compiled_general


def _round_f32r(x: np.ndarray) -> np.ndarray:
    """Round fp32 to the f32r grid (12 dropped mantissa bits, round-nearest).
    Values on the grid are fixed points of the hardware's own rounding."""
    b = np.ascontiguousarray(x, dtype=np.float32).view(np.uint32)
    r = (b + 0x800 + ((b >> 12) & 1)) & np.uint32(0xFFFFF000)
    return r.view(np.float32)


def _pack_zi(zi_core):
    """[BS, H] -> [128, NCH*H]: tile-by-tile, row b0+m*128+p lands in
    partition p, columns (coff+m)*H : (coff+m+1)*H — contiguous per line."""
    blocks = []
    for b0, bt in zip(OFFS, SIZES):
        nm = bt // 128
        blk = zi_core[b0 : b0 + bt].reshape(nm, 128, H)
        blocks.append(blk.transpose(1, 0, 2).reshape(128, nm * H))
    return np.ascontiguousarray(np.concatenate(blocks, axis=1))


def _pack_zjt(zjt_core):
    """[H, BS] -> [128, NK*BS]: tile-by-tile, h=k*128+p lands in partition
    p; per tile the NK k-blocks of bt columns are contiguous per line."""
    blocks = []
    for b0, bt in zip(OFFS, SIZES):
        blk = zjt_core[:, b0 : b0 + bt].reshape(NK, 128, bt)
        blocks.append(blk.transpose(1, 0, 2).reshape(128, NK * bt))
    return np.ascontiguousarray(np.concatenate(blocks, axis=1))


def _kernel_fast(z_i, z_j, R, D):
    import ml_dtypes
    from concourse import bass_utils

    nc = _get_fast()
    bf16 = ml_dtypes.bfloat16

    d2 = np.float32(np.float64(D.flat[0]) ** 2)
    zi_f = np.asarray(z_i, dtype=np.float32)
    if d2 != 1.0:
        zi_f = zi_f * d2  # fold d^2 into z_i (score = sigmoid(d2 * zi.Rzj))
    zi_b = zi_f.astype(bf16)  # [B, H]
    zjt_b = np.asarray(z_j, dtype=np.float32).T.astype(bf16)  # [H, B]
    rt_b = np.asarray(R, dtype=np.float32).T.astype(bf16)  # [H, H]
    rt_p = np.ascontiguousarray(
        rt_b.reshape(NK, 128, H).transpose(1, 0, 2).reshape(128, NK * H)
    )

    in_maps = []
    for c in range(N_CORES):
        sl = slice(c * BS, (c + 1) * BS)
        in_maps.append(
            {
                "zi": _pack_zi(zi_b[sl]),
                "zjt": _pack_zjt(zjt_b[:, sl]),
                "rt": rt_p,
            }
        )

    res = bass_utils.run_bass_kernel_spmd(nc, in_maps, core_ids=list(range(N_CORES)))
    global last_result
    last_result = res
    out = np.empty((B, R_SE), dtype=np.float32)
    for c in range(N_CORES):
        sig = np.asarray(res.results[c]["out"])  # [128, NCH], sig[p,ch]=row ch*128+p
        rows = np.ascontiguousarray(sig.T).reshape(BS)
        out[c * BS : (c + 1) * BS, :] = rows[:, None]
    return out


def _kernel_general(z_i, z_j, R, D):
    from concourse import bass_utils

    nc = _get_general()

    z_i = np.asarray(z_i, dtype=np.float32)
    z_j = np.asarray(z_j, dtype=np.float32)
    zit = np.ascontiguousarray(z_i.T)  # [H, B]
    zjt = _round_f32r(np.ascontiguousarray(z_j.T))
    rt = _round_f32r(np.asarray(R, dtype=np.float32).T)
    d2 = np.asarray(D, dtype=np.float32)
    d2t = _round_f32r((d2 * d2).T)

    in_maps = []
    for c in range(N_CORES):
        sl = slice(c * BS, (c + 1) * BS)
        in_maps.append(
            {
                "zit": np.ascontiguousarray(zit[:, sl]),
                "zjt": np.ascontiguousarray(zjt[:, sl]),
                "rt": rt,
                "d2t": d2t,
            }
        )

    res = bass_utils.run_bass_kernel_spmd(nc, in_maps, core_ids=list(range(N_CORES)))
    global last_result
    last_result = res
    out = np.empty((B, R_SE), dtype=np.float32)
    for c in range(N_CORES):
        out[c * BS : (c + 1) * BS] = res.results[c]["out"]
    return out


def kernel(z_i: np.ndarray, z_j: np.ndarray, R: np.ndarray, D: np.ndarray, **extra):
    D = np.asarray(D)
    if D.size and np.all(D == D.flat[0]):
        return _kernel_fast(z_i, z_j, R, D)
    return _kernel_general(z_i, z_j, R, D)


last_result = None


def _install_ntff_shim():
    """Provide antenv.axon_hooks (absent from this image) so that
    run_bass_kernel_spmd(trace=True) can capture NTFF profiles through
    the axon PJRT .so. No-op if anything is missing."""
    import types
    import contextlib
    import ctypes

    try:
        import antenv
        import antenv.axon_hooks  # noqa: F401

        return  # already present
    except ImportError:
        pass

    so_path = "/opt/axon/libaxon_pjrt.so"
    try:
        lib = ctypes.CDLL(so_path)
    except OSError:
        return
    if not hasattr(lib, "axon_start_nrt_profile"):
        return
    lib.axon_start_nrt_profile.argtypes = [
        ctypes.POINTER(ctypes.c_int64),
        ctypes.c_size_t,
    ]
    lib.axon_start_nrt_profile.restype = ctypes.c_int64
    lib.axon_stop_nrt_profile.argtypes = [ctypes.c_char_p]
    lib.axon_stop_nrt_profile.restype = ctypes.c_int64

    @contextlib.contextmanager
    def _hook(output_dir, device_ids):
        import jax

        jax.devices()
        if device_ids:
            ids = (ctypes.c_int64 * len(device_ids))(*device_ids)
            rc = lib.axon_start_nrt_profile(ids, len(device_ids))
        else:
            rc = lib.axon_start_nrt_profile(None, 0)
        if rc != 0:
            raise RuntimeError(f"axon_start_nrt_profile rc={rc}")
        try:
            yield
        finally:
            n = lib.axon_stop_nrt_profile(str(output_dir).encode())
            print(f"ntff profile: {n} file(s) written to {output_dir}", file=sys.stderr)

    mod = types.ModuleType("antenv.axon_hooks")
    mod.get_axon_ntff_profile_hook = lambda: _hook
    mod.set_axon_ntff_profile_hook = lambda h: None
    sys.modules["antenv.axon_hooks"] = mod
    antenv.axon_hooks = mod


_install_ntff_shim()

